# revision 9
# baseline (speedup 1.0000x reference)
"""Trainium2 Bass kernel for nn_DyMRconv (gnn_message_passing).

Full inputs in, full output out. Data-parallel over batch: 16 samples -> 8
NeuronCores x 2 samples. BatchNorm batch stats via on-device AllReduce.

Algorithm notes (validated against the reference in numpy):
  - The reference's two shift loops are identical (both roll axis 2 with the
    same shift values), so mask_sum doubles and x_j is unchanged; the
    x_weighted comparison is scale-invariant, so single-count sums suffice.
  - s=0 contributes nothing to x_j and a per-sample constant to mask_sum.
  - dist/mask for shift 56-s is a row-roll of dist/mask for shift s, so only
    7 unique shift distances {4,...,28} need the expensive channel reduction.
  - x_j = relu(max_s(roll(x, s) + maskneg_s) - x) with maskneg in {0, -1e30}.
"""

import numpy as np
import ml_dtypes

B, C, H, W = 16, 128, 56, 56
HW = H * W  # 3136
OUT = 384
N_CORES = 8
PER_CORE = B // N_CORES  # 2
UNIQ = [4, 8, 12, 16, 20, 24, 28]
BIG = 1e30
BN_EPS = 1e-5
NTOT = float(B * HW)

_cache = {}


def _roll_regions(r, total=HW):
    """out[p] = in[(p - r) mod total] as list of (out_off, length, in_off)."""
    r = r % total
    if r == 0:
        return [(0, total, 0)]
    return [(0, r, total - r), (r, total - r, 0)]


def build_program(n_reps=1):
    import contextlib
    import concourse.bass as bass
    import concourse.tile as tile
    import concourse.mybir as mybir
    from concourse import bacc

    f32 = mybir.dt.float32
    bf16 = mybir.dt.bfloat16
    Alu = mybir.AluOpType
    Act = mybir.ActivationFunctionType
    AxisX = mybir.AxisListType.X

    nc = bacc.Bacc("TRN2", target_bir_lowering=False, debug=False,
                   num_devices=N_CORES)

    x_d = nc.dram_tensor("x", [PER_CORE, C, HW], f32, kind="ExternalInput").ap()
    wt_d = nc.dram_tensor("wT", [3 * C, OUT], bf16, kind="ExternalInput").ap()
    cb_d = nc.dram_tensor("cb", [OUT], f32, kind="ExternalInput").ap()
    gam_d = nc.dram_tensor("gam", [OUT], f32, kind="ExternalInput").ap()
    bet_d = nc.dram_tensor("bet", [OUT], f32, kind="ExternalInput").ap()
    y_d = nc.dram_tensor("y", [PER_CORE, OUT, HW], f32, kind="ExternalOutput").ap()
    cc_in = nc.dram_tensor("cc_in", [2, OUT], f32).ap()
    cc_out = nc.dram_tensor("cc_out", [2, OUT], f32, addr_space="Shared").ap()
    # DRAM scratch for partition-broadcasts (SBUF sources must have nonzero
    # partition step, DRAM sources may replicate)
    mnr_d = nc.dram_tensor("mnr_d", [8, HW], bf16).ap()
    xw_d = nc.dram_tensor("xw_d", [HW], bf16).ap()
    thr_d = nc.dram_tensor("thr_d", [1], f32).ap()

    def dram_bcast(dst_ap, row_ap, scratch_ap, parts):
        """dst[p, :] = row[:] for all p, bounced through DRAM scratch."""
        nc.sync.dma_start(out=scratch_ap, in_=row_ap)
        rep = bass.AP(tensor=scratch_ap.tensor, offset=scratch_ap.offset,
                      ap=[[0, parts]] + [list(d) for d in scratch_ap.ap])
        nc.sync.dma_start(out=dst_ap, in_=rep)

    CH = [(i * 512, min(512, HW - i * 512)) for i in range((HW + 511) // 512)]

    with tile.TileContext(nc) as tc:
        with contextlib.ExitStack() as ctx:
            consts = ctx.enter_context(tc.tile_pool(name="consts", bufs=1))
            xp = ctx.enter_context(tc.tile_pool(name="xp", bufs=1))
            xbp = ctx.enter_context(tc.tile_pool(name="xbp", bufs=2))
            scratch = ctx.enter_context(tc.tile_pool(name="scratch", bufs=2))
            maskp = ctx.enter_context(tc.tile_pool(name="maskp", bufs=1))
            bcp = ctx.enter_context(tc.tile_pool(name="bcp", bufs=3))
            xwbp = ctx.enter_context(tc.tile_pool(name="xwbp", bufs=1))
            mp = ctx.enter_context(tc.tile_pool(name="mp", bufs=1))
            featp = ctx.enter_context(tc.tile_pool(name="featp", bufs=1))
            yp = ctx.enter_context(tc.tile_pool(name="yp", bufs=1))
            outp = ctx.enter_context(tc.tile_pool(name="outp", bufs=3))
            smalls = ctx.enter_context(tc.tile_pool(name="smalls", bufs=4))
            statp = ctx.enter_context(tc.tile_pool(name="statp", bufs=1))
            bigps = ctx.enter_context(tc.tile_pool(name="bigps", bufs=1, space="PSUM"))
            convps = ctx.enter_context(tc.tile_pool(name="convps", bufs=1, space="PSUM"))

            # ---- constants ----
            wt_sb = consts.tile([C, 3, OUT], bf16)     # lhsT: [k, ko, m]
            nc.sync.dma_start(out=wt_sb[:], in_=wt_d.rearrange(
                "(ko k) o -> k ko o", k=C))
            cb_sb = consts.tile([C, 3], f32)
            nc.sync.dma_start(out=cb_sb[:], in_=bass.AP(
                tensor=cb_d.tensor, offset=0, ap=[[1, C], [C, 3]]))
            gam_sb = consts.tile([C, 3], f32)
            nc.sync.dma_start(out=gam_sb[:], in_=bass.AP(
                tensor=gam_d.tensor, offset=0, ap=[[1, C], [C, 3]]))
            bet_sb = consts.tile([C, 3], f32)
            nc.sync.dma_start(out=bet_sb[:], in_=bass.AP(
                tensor=bet_d.tensor, offset=0, ap=[[1, C], [C, 3]]))
            # dist-row selector weights: wsel[:, j, :] has ones in column j
            wsel = consts.tile([C, 8, 8], f32)
            nc.vector.memset(wsel[:], 0.0)
            for j in range(8):
                nc.vector.memset(wsel[:, j, j:j + 1], 1.0)
            # mask_sum reducer: [14,1] ones, row 0 zero (row 0 = norm garbage)
            ones14 = consts.tile([14, 1], bf16)
            nc.vector.memset(ones14[:], 1.0)
            nc.vector.memset(ones14[0:1, :], 0.0)

            for rep in range(n_reps):
                ysum_parts = [statp.tile([C, 2 * len(CH)], f32, tag=f"ysp{m}",
                                         name=f"ysp{m}") for m in range(3)]
                ysq_parts = [statp.tile([C, 2 * len(CH)], f32, tag=f"yqp{m}",
                                        name=f"yqp{m}") for m in range(3)]
                y_sb = [[yp.tile([C, HW], bf16, tag=f"y{b}{m}", name=f"y{b}{m}")
                         for m in range(3)] for b in range(PER_CORE)]

                for b in range(PER_CORE):
                    x_sb = xp.tile([C, HW], f32, tag="x")
                    nc.sync.dma_start(out=x_sb[:], in_=x_d[b])
                    x_bf = xbp.tile([C, HW], bf16, tag="xbf")
                    nc.vector.tensor_copy(out=x_bf[:], in_=x_sb[:])

                    # ---- dist phase: psum rows = [norm, s=4..28] ----
                    pd = bigps.tile([8, 3584], f32, tag="bigpsum")
                    for j in range(8):
                        diff = scratch.tile([C, HW], f32, tag="diff")
                        if j == 0:
                            # 2D roll by (28, 28)
                            x3 = x_sb[:].rearrange("c (h w) -> c h w", w=W)
                            d3 = diff[:].rearrange("c (h w) -> c h w", w=W)
                            for (ho, hl, hi) in ((0, 28, 28), (28, 28, 0)):
                                for (wo, wl, wi) in ((0, 28, 28), (28, 28, 0)):
                                    nc.vector.tensor_sub(
                                        out=d3[:, ho:ho + hl, wo:wo + wl],
                                        in0=x3[:, ho:ho + hl, wo:wo + wl],
                                        in1=x3[:, hi:hi + hl, wi:wi + wl])
                        else:
                            s = UNIQ[j - 1]
                            for (oo, ln, io) in _roll_regions(s * W):
                                nc.vector.tensor_sub(
                                    out=diff[:, oo:oo + ln],
                                    in0=x_sb[:, oo:oo + ln],
                                    in1=x_sb[:, io:io + ln])
                        adiff = scratch.tile([C, HW], f32, tag="adiff")
                        nc.scalar.activation(out=adiff[:], in_=diff[:], func=Act.Abs)
                        for (cs, cl) in CH:
                            nc.tensor.matmul(
                                out=pd[:, cs:cs + cl], lhsT=wsel[:, j, :],
                                rhs=adiff[:, cs:cs + cl],
                                start=(j == 0), stop=(j == 7))

                    # ---- threshold: thresh = mean - unbiased std of norm row ----
                    stats = smalls.tile([1, 7, 6], f32, tag="bnst")
                    norm_c = pd[0:1, 0:HW].rearrange("p (a b) -> p a b", b=448)
                    for a in range(7):
                        nc.vector.bn_stats(out=stats[:, a, :], in_=norm_c[:, a, :])
                    mv = smalls.tile([1, 2], f32, tag="mv")
                    nc.vector.bn_aggr(out=mv[:], in_=stats[:])
                    varu = smalls.tile([1, 1], f32, tag="varu")
                    nc.vector.tensor_scalar_mul(out=varu[:], in0=mv[:, 1:2],
                                                scalar1=float(HW) / (HW - 1))
                    r0 = smalls.tile([1, 1], f32, tag="r0")
                    nc.scalar.activation(out=r0[:], in_=varu[:], func=Act.Sqrt)
                    ri = smalls.tile([1, 1], f32, tag="ri")
                    nc.vector.reciprocal(out=ri[:], in_=r0[:])
                    t1 = smalls.tile([1, 1], f32, tag="t1")
                    nc.vector.tensor_mul(out=t1[:], in0=varu[:], in1=ri[:])
                    nc.vector.tensor_add(out=t1[:], in0=t1[:], in1=r0[:])
                    nc.vector.tensor_scalar_mul(out=t1[:], in0=t1[:], scalar1=0.5)
                    thresh = smalls.tile([1, 1], f32, tag="thr")
                    nc.vector.tensor_sub(out=thresh[:], in0=mv[:, 0:1], in1=t1[:])
                    mask0 = smalls.tile([1, 1], f32, tag="m0")
                    nc.vector.tensor_scalar(out=mask0[:], in0=thresh[:],
                                            scalar1=0.0, scalar2=None,
                                            op0=Alu.is_gt)
                    thr8 = smalls.tile([8, 1], f32, tag="thr8")
                    dram_bcast(thr8[:], thresh[0:1, :], thr_d[:], 8)

                    # ---- masks (row j = shift 4j; rows 8..13 = paired rolls) ----
                    maskall = maskp.tile([14, HW], bf16, tag="maskall")
                    nc.vector.tensor_scalar(out=maskall[0:8, :], in0=pd[0:8, 0:HW],
                                            scalar1=thr8[:], scalar2=None,
                                            op0=Alu.is_lt)
                    for i, s in enumerate([4, 8, 12, 16, 20, 24]):
                        src = maskall[s // 4:s // 4 + 1, :]
                        dst = maskall[8 + i:9 + i, :]
                        off = s * W
                        nc.sync.dma_start(out=dst[:, 0:HW - off],
                                          in_=src[:, off:HW])
                        nc.sync.dma_start(out=dst[:, HW - off:HW],
                                          in_=src[:, 0:off])
                    mneg = maskp.tile([8, HW], bf16, tag="mneg")
                    nc.vector.tensor_scalar(out=mneg[:], in0=maskall[0:8, :],
                                            scalar1=BIG, scalar2=BIG,
                                            op0=Alu.mult, op1=Alu.subtract)

                    # ---- mask_sum -> x_weighted row ----
                    pm = bigps.tile([1, 3584], f32, tag="bigpsum")
                    for (cs, cl) in CH:
                        nc.tensor.matmul(out=pm[:, cs:cs + cl],
                                         lhsT=ones14[:], rhs=maskall[:, cs:cs + cl],
                                         start=True, stop=True)
                    mmin = smalls.tile([1, 1], f32, tag="mmin")
                    mmax = smalls.tile([1, 1], f32, tag="mmax")
                    nc.vector.tensor_reduce(out=mmin[:], in_=pm[0:1, 0:HW],
                                            axis=AxisX, op=Alu.min)
                    nc.vector.tensor_reduce(out=mmax[:], in_=pm[0:1, 0:HW],
                                            axis=AxisX, op=Alu.max)
                    t04 = smalls.tile([1, 1], f32, tag="t04")
                    nc.vector.tensor_sub(out=t04[:], in0=mmax[:], in1=mmin[:])
                    nc.vector.tensor_scalar_mul(out=t04[:], in0=t04[:], scalar1=0.4)
                    xw_row = maskp.tile([1, HW], bf16, tag="xwrow")
                    nc.vector.tensor_scalar(out=xw_row[:], in0=pm[0:1, 0:HW],
                                            scalar1=mask0[:], scalar2=t04[:],
                                            op0=Alu.add, op1=Alu.is_lt)
                    xw_bc = xwbp.tile([C, HW], bf16, tag="xwbc")
                    dram_bcast(xw_bc[:], xw_row[0:1, :], xw_d[:], C)
                    xwx_bf = featp.tile([C, HW], bf16, tag="xwx")
                    nc.vector.tensor_mul(out=xwx_bf[:], in0=x_bf[:], in1=xw_bc[:])

                    # ---- M-phase: M = max over shifts of (roll(x,s) + maskneg_s) ----
                    M = mp.tile([C, HW], bf16, tag="M")
                    tmp = mp.tile([C, HW], bf16, tag="tmp")
                    first = True
                    for j in range(1, 8):
                        mb = bcp.tile([C, HW], bf16, tag="mnbc")
                        dram_bcast(mb[:], mneg[j:j + 1, :], mnr_d[j], C)
                        sps = [4 * j] if j == 7 else [4 * j, 56 - 4 * j]
                        for sp in sps:
                            r = sp * W
                            dst = M if first else tmp
                            if sp <= 28:
                                nc.vector.tensor_add(out=dst[:, 0:r],
                                                     in0=x_bf[:, HW - r:HW],
                                                     in1=mb[:, 0:r])
                                nc.vector.tensor_add(out=dst[:, r:HW],
                                                     in0=x_bf[:, 0:HW - r],
                                                     in1=mb[:, r:HW])
                            else:
                                nc.vector.tensor_add(out=dst[:, 0:r],
                                                     in0=x_bf[:, HW - r:HW],
                                                     in1=mb[:, HW - r:HW])
                                nc.vector.tensor_add(out=dst[:, r:HW],
                                                     in0=x_bf[:, 0:HW - r],
                                                     in1=mb[:, 0:HW - r])
                            if not first:
                                nc.vector.tensor_max(out=M[:], in0=M[:], in1=tmp[:])
                            first = False
                    xj_bf = featp.tile([C, HW], bf16, tag="xj")
                    nc.vector.tensor_sub(out=xj_bf[:], in0=M[:], in1=x_bf[:])
                    nc.vector.tensor_scalar_max(out=xj_bf[:], in0=xj_bf[:],
                                                scalar1=0.0)

                    # ---- 1x1 conv (bf16) + per-chunk channel stats ----
                    feat = [x_bf, xj_bf, xwx_bf]
                    for ci, (cs, cl) in enumerate(CH):
                        for m in range(3):
                            py = convps.tile([C, 512], f32, tag="py")
                            for ko in range(3):
                                nc.tensor.matmul(
                                    out=py[:, 0:cl],
                                    lhsT=wt_sb[:, ko, m * C:(m + 1) * C],
                                    rhs=feat[ko][:, cs:cs + cl],
                                    start=(ko == 0), stop=(ko == 2))
                            idx = b * len(CH) + ci
                            nc.scalar.activation(
                                out=y_sb[b][m][:, cs:cs + cl], in_=py[:, 0:cl],
                                func=Act.Identity, bias=cb_sb[:, m:m + 1],
                                accum_out=ysum_parts[m][:, idx:idx + 1])
                            sq = scratch.tile([C, 512], bf16, tag="sq")
                            nc.scalar.activation(
                                out=sq[:, 0:cl], in_=y_sb[b][m][:, cs:cs + cl],
                                func=Act.Square,
                                accum_out=ysq_parts[m][:, idx:idx + 1])

                # ---- global BN stats (AllReduce over 8 cores) ----
                for m in range(3):
                    ys = smalls.tile([C, 1], f32, tag=f"ys{m}")
                    nc.vector.reduce_sum(out=ys[:], in_=ysum_parts[m][:], axis=AxisX)
                    nc.sync.dma_start(out=cc_in[0, m * C:(m + 1) * C], in_=ys[:, 0])
                    yq = smalls.tile([C, 1], f32, tag=f"yq{m}")
                    nc.vector.reduce_sum(out=yq[:], in_=ysq_parts[m][:], axis=AxisX)
                    nc.sync.dma_start(out=cc_in[1, m * C:(m + 1) * C], in_=yq[:, 0])
                nc.gpsimd.collective_compute(
                    "AllReduce", mybir.AluOpType.add,
                    replica_groups=[list(range(N_CORES))],
                    ins=[cc_in[:]], outs=[cc_out[:]])
                sums = statp.tile([C, 3], f32, tag="sums")
                nc.sync.dma_start(out=sums[:], in_=bass.AP(
                    tensor=cc_out.tensor, offset=0, ap=[[1, C], [C, 3]]))
                sqs = statp.tile([C, 3], f32, tag="sqs")
                nc.sync.dma_start(out=sqs[:], in_=bass.AP(
                    tensor=cc_out.tensor, offset=OUT, ap=[[1, C], [C, 3]]))
                mu = statp.tile([C, 3], f32, tag="mu")
                nc.vector.tensor_scalar_mul(out=mu[:], in0=sums[:], scalar1=1.0 / NTOT)
                ve = statp.tile([C, 3], f32, tag="ve")
                nc.vector.tensor_scalar_mul(out=ve[:], in0=sqs[:], scalar1=1.0 / NTOT)
                t2 = statp.tile([C, 3], f32, tag="t2")
                nc.vector.tensor_mul(out=t2[:], in0=mu[:], in1=mu[:])
                nc.vector.tensor_sub(out=ve[:], in0=ve[:], in1=t2[:])
                nc.vector.tensor_scalar_add(out=ve[:], in0=ve[:], scalar1=BN_EPS)
                rr = statp.tile([C, 3], f32, tag="rr")
                nc.scalar.activation(out=rr[:], in_=ve[:], func=Act.Sqrt)
                rv = statp.tile([C, 3], f32, tag="rv")
                nc.vector.reciprocal(out=rv[:], in_=rr[:])
                nc.vector.tensor_mul(out=rv[:], in0=ve[:], in1=rv[:])
                nc.vector.tensor_add(out=rv[:], in0=rv[:], in1=rr[:])
                nc.vector.tensor_scalar_mul(out=rv[:], in0=rv[:], scalar1=0.5)
                rstd = statp.tile([C, 3], f32, tag="rstd")
                nc.vector.reciprocal(out=rstd[:], in_=rv[:])
                a_sb = statp.tile([C, 3], f32, tag="asb")
                nc.vector.tensor_mul(out=a_sb[:], in0=rstd[:], in1=gam_sb[:])
                b_sb = statp.tile([C, 3], f32, tag="bsb")
                nc.vector.tensor_mul(out=b_sb[:], in0=mu[:], in1=a_sb[:])
                nc.vector.tensor_sub(out=b_sb[:], in0=bet_sb[:], in1=b_sb[:])

                # ---- BN + gelu + store ----
                y3 = y_d.rearrange("b (mo k) hw -> b mo k hw", k=C)
                for b in range(PER_CORE):
                    for m in range(3):
                        for (cs, cl) in CH:
                            ot = outp.tile([C, 512], f32, tag="out")
                            nc.scalar.activation(out=ot[:, 0:cl],
                                                 in_=y_sb[b][m][:, cs:cs + cl],
                                                 func=Act.Gelu,
                                                 bias=b_sb[:, m:m + 1],
                                                 scale=a_sb[:, m:m + 1])
                            nc.sync.dma_start(out=y3[b, m, :, cs:cs + cl],
                                              in_=ot[:, 0:cl])

    nc.compile()
    return nc


def _prep_inputs(x, conv_w, conv_b, gamma, beta):
    wt = np.ascontiguousarray(conv_w.T).astype(ml_dtypes.bfloat16)
    maps = []
    for c in range(N_CORES):
        xs = np.ascontiguousarray(
            x[c * PER_CORE:(c + 1) * PER_CORE].reshape(PER_CORE, C, HW))
        maps.append({"x": xs, "wT": wt, "cb": conv_b.astype(np.float32),
                     "gam": gamma.astype(np.float32),
                     "bet": beta.astype(np.float32)})
    return maps


def run_on_cores(nc, in_maps):
    from concourse.bass_utils import run_bass_kernel_spmd
    return run_bass_kernel_spmd(nc, in_maps, list(range(N_CORES)))


def kernel(x, conv_w, conv_b, gamma, beta):
    x = np.asarray(x, dtype=np.float32)
    conv_w = np.asarray(conv_w, dtype=np.float32)
    conv_b = np.asarray(conv_b, dtype=np.float32)
    gamma = np.asarray(gamma, dtype=np.float32)
    beta = np.asarray(beta, dtype=np.float32)
    if "nc" not in _cache:
        _cache["nc"] = build_program(1)
    nc = _cache["nc"]
    in_maps = _prep_inputs(x, conv_w, conv_b, gamma, beta)
    res = run_on_cores(nc, in_maps)
    out = np.concatenate([res.results[i]["y"].reshape(PER_CORE, OUT, H, W)
                          for i in range(N_CORES)], axis=0)
    return out.astype(np.float32)


# revision 40
# speedup vs baseline: 180.9563x; 180.9563x over previous
"""Trainium2 Bass kernel for nn_DyMRconv (gnn_message_passing).

Full inputs in, full output out. Data-parallel over batch: 16 samples -> 8
NeuronCores x 2 samples. BatchNorm batch stats via on-device AllReduce.

Algorithm notes (validated against the reference in numpy):
  - The reference's two shift loops are identical (both roll axis 2 with the
    same shift values), so mask_sum doubles and x_j is unchanged; the
    x_weighted comparison is scale-invariant, so single-count sums suffice.
  - s=0 contributes nothing to x_j and a per-sample constant to mask_sum.
  - dist/mask for shift 56-s is a row-roll of dist/mask for shift s, so only
    7 unique shift distances {4,...,28} need the expensive channel reduction.
  - x_j = relu(max_s(roll(x, s) + maskneg_s) - x) with maskneg in {0, -1e30}.
"""

import numpy as np
import ml_dtypes

B, C, H, W = 16, 128, 56, 56
HW = H * W  # 3136
OUT = 384
N_CORES = 8
PER_CORE = B // N_CORES  # 2
UNIQ = [4, 8, 12, 16, 20, 24, 28]
BIG = 1e30
BN_EPS = 1e-5
NTOT = float(B * HW)

_cache = {}


def _roll_regions(r, total=HW):
    """out[p] = in[(p - r) mod total] as list of (out_off, length, in_off)."""
    r = r % total
    if r == 0:
        return [(0, total, 0)]
    return [(0, r, total - r), (r, total - r, 0)]


def build_program(n_reps=1, sim_mode=False, interleave=False, thresh_inloop=False, use_collective=True):
    import contextlib
    import concourse.bass as bass
    import concourse.tile as tile
    import concourse.mybir as mybir
    from concourse import bacc

    f32 = mybir.dt.float32
    bf16 = mybir.dt.bfloat16
    Alu = mybir.AluOpType
    Act = mybir.ActivationFunctionType
    AxisX = mybir.AxisListType.X

    nc = bacc.Bacc("TRN2", target_bir_lowering=False, debug=False,
                   num_devices=1 if sim_mode else N_CORES)

    x_d = nc.dram_tensor("x", [PER_CORE, C, HW], f32, kind="ExternalInput").ap()
    wt_d = nc.dram_tensor("wT", [3 * C, OUT], bf16, kind="ExternalInput").ap()
    cb_d = nc.dram_tensor("cb", [OUT], f32, kind="ExternalInput").ap()
    gam_d = nc.dram_tensor("gam", [OUT], f32, kind="ExternalInput").ap()
    bet_d = nc.dram_tensor("bet", [OUT], f32, kind="ExternalInput").ap()
    y_d = nc.dram_tensor("y", [PER_CORE, OUT, HW], f32, kind="ExternalOutput").ap()
    cc_in = nc.dram_tensor("cc_in", [2, OUT], f32).ap()
    cc_out = nc.dram_tensor("cc_out", [2, OUT], f32, addr_space="Shared").ap()
    # DRAM scratch for partition-broadcasts (SBUF sources must have nonzero
    # partition step, DRAM sources may replicate)
    mnr_d = nc.dram_tensor("mnr_d", [8, HW], bf16).ap()
    xw_d = nc.dram_tensor("xw_d", [HW], bf16).ap()
    thr_d = nc.dram_tensor("thr_d", [1], f32).ap()

    def dram_bcast(dst_ap, row_ap, scratch_ap, parts):
        """dst[p, :] = row[:] for all p, bounced through DRAM scratch."""
        nc.sync.dma_start(out=scratch_ap, in_=row_ap)
        rep = bass.AP(tensor=scratch_ap.tensor, offset=scratch_ap.offset,
                      ap=[[0, parts]] + [list(d) for d in scratch_ap.ap])
        nc.sync.dma_start(out=dst_ap, in_=rep)

    CH = [(i * 512, min(512, HW - i * 512)) for i in range((HW + 511) // 512)]

    with tile.TileContext(nc) as tc:
        with contextlib.ExitStack() as ctx:
            consts = ctx.enter_context(tc.tile_pool(name="consts", bufs=1))
            xp = ctx.enter_context(tc.tile_pool(name="xp", bufs=2))
            xbp = ctx.enter_context(tc.tile_pool(name="xbp", bufs=2))
            scratch = ctx.enter_context(tc.tile_pool(name="scratch", bufs=2))
            maskp = ctx.enter_context(tc.tile_pool(name="maskp", bufs=1))
            xwbp = ctx.enter_context(tc.tile_pool(name="xwbp", bufs=1))
            bcp = ctx.enter_context(tc.tile_pool(name="bcp", bufs=3))
            mp = ctx.enter_context(tc.tile_pool(name="mp", bufs=1))
            featp = ctx.enter_context(tc.tile_pool(name="featp", bufs=1))
            yp = ctx.enter_context(tc.tile_pool(name="yp", bufs=1))
            smalls = ctx.enter_context(tc.tile_pool(name="smalls", bufs=2))
            statp = ctx.enter_context(tc.tile_pool(name="statp", bufs=1))
            bigps = ctx.enter_context(tc.tile_pool(name="bigps", bufs=1, space="PSUM"))
            convps = ctx.enter_context(tc.tile_pool(name="convps", bufs=1, space="PSUM"))

            # ---- constants ----
            wt_sb = consts.tile([C, 3, OUT], bf16)     # lhsT: [k, ko, m]
            nc.sync.dma_start(out=wt_sb[:], in_=wt_d.rearrange(
                "(ko k) o -> k ko o", k=C))
            cb_sb = consts.tile([C, 3], f32)
            nc.sync.dma_start(out=cb_sb[:], in_=bass.AP(
                tensor=cb_d.tensor, offset=0, ap=[[1, C], [C, 3]]))
            gam_sb = consts.tile([C, 3], f32)
            nc.sync.dma_start(out=gam_sb[:], in_=bass.AP(
                tensor=gam_d.tensor, offset=0, ap=[[1, C], [C, 3]]))
            bet_sb = consts.tile([C, 3], f32)
            nc.sync.dma_start(out=bet_sb[:], in_=bass.AP(
                tensor=bet_d.tensor, offset=0, ap=[[1, C], [C, 3]]))
            # dist-row selector weights: wsel[:, j, :] has ones in column j
            wsel = consts.tile([C, 8, 8], f32)
            nc.vector.memset(wsel[:], 0.0)
            for j in range(8):
                nc.vector.memset(wsel[:, j, j:j + 1], 1.0)
            # mask_sum chunk->row selectors: ms_sel[:, c, :] ones in col c,
            # rows 1..13 only (row 0 of maskall is norm garbage)
            ms_sel = consts.tile([14, 7, 8], bf16)
            nc.vector.memset(ms_sel[:], 0.0)
            for c in range(7):
                nc.vector.memset(ms_sel[:, c, c:c + 1], 1.0)
                nc.vector.memset(ms_sel[0:1, c, c:c + 1], 0.0)

            for rep in range(n_reps):
                ysum_parts = [statp.tile([C, 2 * len(CH)], f32, tag=f"ysp{m}",
                                         name=f"ysp{m}") for m in range(3)]
                ysq_parts = [statp.tile([C, 2 * len(CH)], f32, tag=f"yqp{m}",
                                        name=f"yqp{m}") for m in range(3)]
                y_sb = [[yp.tile([C, HW], bf16, tag=f"y{b}{m}", name=f"y{b}{m}")
                         for m in range(3)] for b in range(PER_CORE)]

                def stage_load(b, st):
                    st["x_sb"] = xp.tile([C, HW], f32, tag="x", name="x_sb")
                    nc.sync.dma_start(out=st["x_sb"][:], in_=x_d[b])
                    st["x_bf"] = xbp.tile([C, HW], bf16, tag="xbf", name="x_bf")
                    nc.scalar.copy(out=st["x_bf"][:], in_=st["x_sb"][:])

                def stage_dist(b, st):
                    x_sb, x_bf = st["x_sb"], st["x_bf"]
                    # dist psum rows = [norm, s=4..28]; threshold chain is
                    # emitted right after the norm round so it overlaps the
                    # remaining shift rounds
                    pd = bigps.tile([8, 3584], f32, tag="bigpsum", name="pd")
                    for j in range(8):
                        diff = scratch.tile([C, HW], f32, tag="diff", name="diff")
                        if j == 0:
                            x3 = x_sb[:].rearrange("c (h w) -> c h w", w=W)
                            d3 = diff[:].rearrange("c (h w) -> c h w", w=W)
                            for (ho, hl, hi) in ((0, 28, 28), (28, 28, 0)):
                                for (wo, wl, wi) in ((0, 28, 28), (28, 28, 0)):
                                    nc.vector.tensor_sub(
                                        out=d3[:, ho:ho + hl, wo:wo + wl],
                                        in0=x3[:, ho:ho + hl, wo:wo + wl],
                                        in1=x3[:, hi:hi + hl, wi:wi + wl])
                        else:
                            s = UNIQ[j - 1]
                            for (oo, ln, io) in _roll_regions(s * W):
                                nc.vector.tensor_sub(
                                    out=diff[:, oo:oo + ln],
                                    in0=x_sb[:, oo:oo + ln],
                                    in1=x_sb[:, io:io + ln])
                        adiff = scratch.tile([C, HW], f32, tag="adiff",
                                             name="adiff")
                        nc.scalar.activation(out=adiff[:], in_=diff[:],
                                             func=Act.Abs)
                        for (cs, cl) in CH:
                            nc.tensor.matmul(
                                out=pd[:, cs:cs + cl], lhsT=wsel[:, j, :],
                                rhs=adiff[:, cs:cs + cl],
                                start=(j == 0), stop=(j == 7))
                        if (j == 0 and thresh_inloop) or (j == 7 and not thresh_inloop):
                            # threshold: thresh = mean - unbiased std of norm
                            stats = smalls.tile([1, 7, 6], f32, tag="bnst",
                                                name="stats")
                            norm_c = pd[0:1, 0:HW].rearrange(
                                "p (a b) -> p a b", b=448)
                            for a in range(7):
                                nc.vector.bn_stats(out=stats[:, a, :],
                                                   in_=norm_c[:, a, :])
                            mv = smalls.tile([1, 2], f32, tag="mv", name="mv")
                            nc.vector.bn_aggr(out=mv[:], in_=stats[:])
                            varu = smalls.tile([1, 1], f32, tag="varu",
                                               name="varu")
                            nc.vector.tensor_scalar_mul(
                                out=varu[:], in0=mv[:, 1:2],
                                scalar1=float(HW) / (HW - 1))
                            r0 = smalls.tile([1, 1], f32, tag="r0", name="r0")
                            nc.scalar.activation(out=r0[:], in_=varu[:],
                                                 func=Act.Sqrt)
                            ri = smalls.tile([1, 1], f32, tag="ri", name="ri")
                            nc.vector.reciprocal(out=ri[:], in_=r0[:])
                            t1 = smalls.tile([1, 1], f32, tag="t1", name="t1")
                            nc.vector.tensor_mul(out=t1[:], in0=varu[:],
                                                 in1=ri[:])
                            nc.vector.tensor_add(out=t1[:], in0=t1[:],
                                                 in1=r0[:])
                            nc.vector.tensor_scalar_mul(out=t1[:], in0=t1[:],
                                                        scalar1=0.5)
                            thresh = smalls.tile([1, 1], f32, tag="thr",
                                                 name="thresh")
                            nc.vector.tensor_sub(out=thresh[:], in0=mv[:, 0:1],
                                                 in1=t1[:])
                            mask0 = smalls.tile([1, 1], f32, tag="m0",
                                                name="mask0")
                            nc.vector.tensor_scalar(out=mask0[:],
                                                    in0=thresh[:],
                                                    scalar1=0.0, scalar2=None,
                                                    op0=Alu.is_gt)
                            thr8 = smalls.tile([8, 1], f32, tag="thr8",
                                               name="thr8")
                            dram_bcast(thr8[:], thresh[0:1, :], thr_d[:], 8)
                            st["mask0"] = mask0

                    # masks (row j = shift 4j; rows 8..13 = paired rolls)
                    maskall = maskp.tile([14, HW], bf16, tag="maskall",
                                         name="maskall")
                    nc.vector.tensor_scalar(out=maskall[0:8, :],
                                            in0=pd[0:8, 0:HW],
                                            scalar1=thr8[:], scalar2=None,
                                            op0=Alu.is_lt)
                    for i, s in enumerate([4, 8, 12, 16, 20, 24]):
                        msrc = maskall[s // 4:s // 4 + 1, :]
                        mdst = maskall[8 + i:9 + i, :]
                        off = s * W
                        nc.sync.dma_start(out=mdst[:, 0:HW - off],
                                          in_=msrc[:, off:HW])
                        nc.sync.dma_start(out=mdst[:, HW - off:HW],
                                          in_=msrc[:, 0:off])
                    mneg = maskp.tile([8, HW], bf16, tag="mneg", name="mneg")
                    nc.vector.tensor_scalar(out=mneg[:], in0=maskall[0:8, :],
                                            scalar1=BIG, scalar2=BIG,
                                            op0=Alu.mult, op1=Alu.subtract)
                    st["mneg"] = mneg

                    # mask_sum -> x_weighted: 448-col chunks land on psum rows
                    # 0..6 (selector lhsT) so stats ops run 7 lanes wide
                    pms = convps.tile([8, 448], f32, tag="py", name="pms")
                    for c in range(7):
                        nc.tensor.matmul(out=pms[:, :],
                                         lhsT=ms_sel[:, c, :],
                                         rhs=maskall[:, c * 448:(c + 1) * 448],
                                         start=(c == 0), stop=(c == 6))
                    mmin7 = smalls.tile([7, 1], f32, tag="mmin7", name="mmin7")
                    mmax7 = smalls.tile([7, 1], f32, tag="mmax7", name="mmax7")
                    nc.vector.tensor_reduce(out=mmin7[:], in_=pms[0:7, :],
                                            axis=AxisX, op=Alu.min)
                    nc.vector.tensor_reduce(out=mmax7[:], in_=pms[0:7, :],
                                            axis=AxisX, op=Alu.max)
                    mm17 = smalls.tile([1, 7, 2], f32, tag="mm17", name="mm17")
                    nc.sync.dma_start(out=mm17[:, :, 0], in_=mmin7[:, 0])
                    nc.sync.dma_start(out=mm17[:, :, 1], in_=mmax7[:, 0])
                    mmin = smalls.tile([1, 1], f32, tag="mmin", name="mmin")
                    mmax = smalls.tile([1, 1], f32, tag="mmax", name="mmax")
                    nc.vector.tensor_reduce(out=mmin[:], in_=mm17[:, :, 0],
                                            axis=AxisX, op=Alu.min)
                    nc.vector.tensor_reduce(out=mmax[:], in_=mm17[:, :, 1],
                                            axis=AxisX, op=Alu.max)
                    # xw threshold: msum < 0.4*(mmax-mmin) - mask0
                    t04 = smalls.tile([1, 1], f32, tag="t04", name="t04")
                    nc.vector.tensor_sub(out=t04[:], in0=mmax[:], in1=mmin[:])
                    nc.vector.tensor_scalar_mul(out=t04[:], in0=t04[:],
                                                scalar1=0.4)
                    nc.vector.tensor_sub(out=t04[:], in0=t04[:],
                                         in1=st["mask0"][:])
                    t047 = smalls.tile([7, 1], f32, tag="t047", name="t047")
                    dram_bcast(t047[:], t04[0:1, :], thr_d[:], 7)
                    xw_rows = maskp.tile([7, 448], bf16, tag="xwrows",
                                         name="xw_rows")
                    nc.vector.tensor_scalar(out=xw_rows[:], in0=pms[0:7, :],
                                            scalar1=t047[:], scalar2=None,
                                            op0=Alu.is_lt)
                    nc.sync.dma_start(
                        out=bass.AP(tensor=xw_d.tensor, offset=0,
                                    ap=[[448, 7], [1, 448]]),
                        in_=xw_rows[:])
                    xw_bc = xwbp.tile([C, HW], bf16, tag="xwbc", name="xw_bc")
                    nc.sync.dma_start(out=xw_bc[:], in_=bass.AP(
                        tensor=xw_d.tensor, offset=0,
                        ap=[[0, C], [1, HW]]))
                    xwx_bf = featp.tile([C, HW], bf16, tag="xwx", name="xwx_bf")
                    nc.vector.tensor_mul(out=xwx_bf[:], in0=st["x_bf"][:],
                                         in1=xw_bc[:])
                    st["xwx_bf"] = xwx_bf

                def stage_m(b, st):
                    x_bf = st["x_bf"]
                    M = mp.tile([C, HW], bf16, tag="M", name="M")
                    tmp = mp.tile([C, HW], bf16, tag="tmp", name="tmp")
                    first = True
                    for j in range(1, 8):
                        mb = bcp.tile([C, HW], bf16, tag="mnbc", name="mb")
                        dram_bcast(mb[:], st["mneg"][j:j + 1, :], mnr_d[j], C)
                        sps = [4 * j] if j == 7 else [4 * j, 56 - 4 * j]
                        for sp in sps:
                            r = sp * W
                            dst = M if first else tmp
                            if sp <= 28:
                                nc.vector.tensor_add(out=dst[:, 0:r],
                                                     in0=x_bf[:, HW - r:HW],
                                                     in1=mb[:, 0:r])
                                nc.vector.tensor_add(out=dst[:, r:HW],
                                                     in0=x_bf[:, 0:HW - r],
                                                     in1=mb[:, r:HW])
                            else:
                                nc.vector.tensor_add(out=dst[:, 0:r],
                                                     in0=x_bf[:, HW - r:HW],
                                                     in1=mb[:, HW - r:HW])
                                nc.vector.tensor_add(out=dst[:, r:HW],
                                                     in0=x_bf[:, 0:HW - r],
                                                     in1=mb[:, 0:HW - r])
                            if not first:
                                nc.vector.tensor_max(out=M[:], in0=M[:],
                                                     in1=tmp[:])
                            first = False
                    xj_bf = featp.tile([C, HW], bf16, tag="xj", name="xj_bf")
                    nc.vector.tensor_sub(out=xj_bf[:], in0=M[:], in1=x_bf[:])
                    nc.vector.tensor_scalar_max(out=xj_bf[:], in0=xj_bf[:],
                                                scalar1=0.0)
                    st["xj_bf"] = xj_bf

                def stage_conv(b, st):
                    feat = [st["x_bf"], st["xj_bf"], st["xwx_bf"]]
                    KO_ORDER = (0, 2, 1)  # xj (ko=1) last: ready latest
                    for ci, (cs, cl) in enumerate(CH):
                        for m in range(3):
                            py = convps.tile([C, 512], f32, tag="py", name="py")
                            for ki, ko in enumerate(KO_ORDER):
                                nc.tensor.matmul(
                                    out=py[:, 0:cl],
                                    lhsT=wt_sb[:, ko, m * C:(m + 1) * C],
                                    rhs=feat[ko][:, cs:cs + cl],
                                    start=(ki == 0), stop=(ki == 2))
                            idx = b * len(CH) + ci
                            nc.scalar.activation(
                                out=y_sb[b][m][:, cs:cs + cl], in_=py[:, 0:cl],
                                func=Act.Identity, bias=cb_sb[:, m:m + 1],
                                accum_out=ysum_parts[m][:, idx:idx + 1])
                    for m in range(3):
                        sq = scratch.tile([C, HW], bf16, tag="sq", name="sq", bufs=1)
                        nc.scalar.activation(
                            out=sq[:], in_=y_sb[b][m][:], func=Act.Square,
                            accum_out=ysq_parts[m][:, b:b + 1])

                # software-pipelined emission: dist(1) before M(0) keeps the
                # PE fed with matmuls while the DVE runs the max-accumulation
                sts = [dict() for _ in range(PER_CORE)]
                if interleave:
                    stage_load(0, sts[0])
                    stage_dist(0, sts[0])
                    stage_load(1, sts[1])
                    stage_dist(1, sts[1])
                    stage_m(0, sts[0])
                    stage_conv(0, sts[0])
                    stage_m(1, sts[1])
                    stage_conv(1, sts[1])
                else:
                    for b in range(PER_CORE):
                        stage_load(b, sts[b])
                        stage_dist(b, sts[b])
                        stage_m(b, sts[b])
                        stage_conv(b, sts[b])

                # ---- global BN stats (AllReduce over 8 cores) ----
                for m in range(3):
                    ys = smalls.tile([C, 1], f32, tag=f"ys{m}")
                    nc.vector.reduce_sum(out=ys[:], in_=ysum_parts[m][:], axis=AxisX)
                    nc.sync.dma_start(out=cc_in[0, m * C:(m + 1) * C], in_=ys[:, 0])
                    yq = smalls.tile([C, 1], f32, tag=f"yq{m}")
                    nc.vector.reduce_sum(out=yq[:], in_=ysq_parts[m][:], axis=AxisX)
                    nc.sync.dma_start(out=cc_in[1, m * C:(m + 1) * C], in_=yq[:, 0])
                if sim_mode or not use_collective:
                    # TimelineSim has no collective support; timing-equivalent
                    # stand-in (collective latency not modeled either way)
                    nc.sync.dma_start(out=cc_out[:], in_=cc_in[:])
                else:
                    nc.gpsimd.collective_compute(
                        "AllReduce", mybir.AluOpType.add,
                        replica_groups=[list(range(N_CORES))],
                        ins=[cc_in[:]], outs=[cc_out[:]])
                sums = statp.tile([C, 3], f32, tag="sums")
                nc.sync.dma_start(out=sums[:], in_=bass.AP(
                    tensor=cc_out.tensor, offset=0, ap=[[1, C], [C, 3]]))
                sqs = statp.tile([C, 3], f32, tag="sqs")
                nc.sync.dma_start(out=sqs[:], in_=bass.AP(
                    tensor=cc_out.tensor, offset=OUT, ap=[[1, C], [C, 3]]))
                mu = statp.tile([C, 3], f32, tag="mu")
                nc.vector.tensor_scalar_mul(out=mu[:], in0=sums[:], scalar1=1.0 / NTOT)
                ve = statp.tile([C, 3], f32, tag="ve")
                nc.vector.tensor_scalar_mul(out=ve[:], in0=sqs[:], scalar1=1.0 / NTOT)
                t2 = statp.tile([C, 3], f32, tag="t2")
                nc.vector.tensor_mul(out=t2[:], in0=mu[:], in1=mu[:])
                nc.vector.tensor_sub(out=ve[:], in0=ve[:], in1=t2[:])
                nc.vector.tensor_scalar_add(out=ve[:], in0=ve[:], scalar1=BN_EPS)
                rr = statp.tile([C, 3], f32, tag="rr")
                nc.scalar.activation(out=rr[:], in_=ve[:], func=Act.Sqrt)
                rv = statp.tile([C, 3], f32, tag="rv")
                nc.vector.reciprocal(out=rv[:], in_=rr[:])
                nc.vector.tensor_mul(out=rv[:], in0=ve[:], in1=rv[:])
                nc.vector.tensor_add(out=rv[:], in0=rv[:], in1=rr[:])
                nc.vector.tensor_scalar_mul(out=rv[:], in0=rv[:], scalar1=0.5)
                rstd = statp.tile([C, 3], f32, tag="rstd")
                nc.vector.reciprocal(out=rstd[:], in_=rv[:])
                a_sb = statp.tile([C, 3], f32, tag="asb")
                nc.vector.tensor_mul(out=a_sb[:], in0=rstd[:], in1=gam_sb[:])
                b_sb = statp.tile([C, 3], f32, tag="bsb")
                nc.vector.tensor_mul(out=b_sb[:], in0=mu[:], in1=a_sb[:])
                nc.vector.tensor_sub(out=b_sb[:], in0=bet_sb[:], in1=b_sb[:])

                # ---- BN + gelu + store (full rows; staging reuses diff slot) ----
                y3 = y_d.rearrange("b (mo k) hw -> b mo k hw", k=C)
                HH = HW // 2
                for b in range(PER_CORE):
                    for m in range(3):
                        ot = scratch.tile([C, HW], f32, tag="diff", name="ot")
                        for h0 in (0, HH):
                            nc.scalar.activation(out=ot[:, h0:h0 + HH],
                                                 in_=y_sb[b][m][:, h0:h0 + HH],
                                                 func=Act.Gelu,
                                                 bias=b_sb[:, m:m + 1],
                                                 scale=a_sb[:, m:m + 1])
                            nc.sync.dma_start(out=y3[b, m, :, h0:h0 + HH],
                                              in_=ot[:, h0:h0 + HH])

    nc.compile()
    return nc


def _prep_inputs(x, conv_w, conv_b, gamma, beta):
    wt = np.ascontiguousarray(conv_w.T).astype(ml_dtypes.bfloat16)
    maps = []
    for c in range(N_CORES):
        xs = np.ascontiguousarray(
            x[c * PER_CORE:(c + 1) * PER_CORE].reshape(PER_CORE, C, HW))
        maps.append({"x": xs, "wT": wt, "cb": conv_b.astype(np.float32),
                     "gam": gamma.astype(np.float32),
                     "bet": beta.astype(np.float32)})
    return maps


def run_on_cores(nc, in_maps):
    from concourse.bass_utils import run_bass_kernel_spmd
    return run_bass_kernel_spmd(nc, in_maps, list(range(N_CORES)))


def kernel(x, conv_w, conv_b, gamma, beta):
    x = np.asarray(x, dtype=np.float32)
    conv_w = np.asarray(conv_w, dtype=np.float32)
    conv_b = np.asarray(conv_b, dtype=np.float32)
    gamma = np.asarray(gamma, dtype=np.float32)
    beta = np.asarray(beta, dtype=np.float32)
    if "nc" not in _cache:
        _cache["nc"] = build_program(1)
    nc = _cache["nc"]
    in_maps = _prep_inputs(x, conv_w, conv_b, gamma, beta)
    res = run_on_cores(nc, in_maps)
    out = np.concatenate([res.results[i]["y"].reshape(PER_CORE, OUT, H, W)
                          for i in range(N_CORES)], axis=0)
    return out.astype(np.float32)


# revision 53
# speedup vs baseline: 204.3362x; 1.1292x over previous
"""Trainium2 Bass kernel for nn_DyMRconv (gnn_message_passing).

Full inputs in, full output out. Data-parallel over batch: 16 samples -> 8
NeuronCores x 2 samples. BatchNorm batch stats via on-device AllReduce.

Algorithm notes (validated against the reference in numpy):
  - The reference's two shift loops are identical (both roll axis 2 with the
    same shift values), so mask_sum doubles and x_j is unchanged; the
    x_weighted comparison is scale-invariant, so single-count sums suffice.
  - s=0 contributes nothing to x_j and a per-sample constant to mask_sum.
  - dist/mask for shift 56-s is a row-roll of dist/mask for shift s, so only
    7 unique shift distances {4,...,28} need the expensive channel reduction.
  - x_j = relu(max_s(roll(x, s) + maskneg_s) - x) with maskneg in {0, -1e30}.
"""

import numpy as np
import ml_dtypes

B, C, H, W = 16, 128, 56, 56
HW = H * W  # 3136
OUT = 384
N_CORES = 8
PER_CORE = B // N_CORES  # 2
UNIQ = [4, 8, 12, 16, 20, 24, 28]
BIG = 1e30
BN_EPS = 1e-5
NTOT = float(B * HW)

_cache = {}


def _roll_regions(r, total=HW):
    """out[p] = in[(p - r) mod total] as list of (out_off, length, in_off)."""
    r = r % total
    if r == 0:
        return [(0, total, 0)]
    return [(0, r, total - r), (r, total - r, 0)]


def build_program(n_reps=1, sim_mode=False, interleave=False, thresh_inloop=False, use_collective=True):
    import contextlib
    import concourse.bass as bass
    import concourse.tile as tile
    import concourse.mybir as mybir
    from concourse import bacc

    f32 = mybir.dt.float32
    bf16 = mybir.dt.bfloat16
    Alu = mybir.AluOpType
    Act = mybir.ActivationFunctionType
    AxisX = mybir.AxisListType.X

    nc = bacc.Bacc("TRN2", target_bir_lowering=False, debug=False,
                   num_devices=1 if sim_mode else N_CORES)

    x_d = nc.dram_tensor("x", [PER_CORE, C, HW], f32, kind="ExternalInput").ap()
    wt_d = nc.dram_tensor("wT", [3 * C, OUT], bf16, kind="ExternalInput").ap()
    cb_d = nc.dram_tensor("cb", [OUT], f32, kind="ExternalInput").ap()
    gam_d = nc.dram_tensor("gam", [OUT], f32, kind="ExternalInput").ap()
    bet_d = nc.dram_tensor("bet", [OUT], f32, kind="ExternalInput").ap()
    y_d = nc.dram_tensor("y", [PER_CORE, OUT, HW], f32, kind="ExternalOutput").ap()
    cc_in = nc.dram_tensor("cc_in", [2, OUT], f32).ap()
    cc_out = nc.dram_tensor("cc_out", [2, OUT], f32, addr_space="Shared").ap()
    # DRAM scratch for partition-broadcasts (SBUF sources must have nonzero
    # partition step, DRAM sources may replicate)
    mnr_d = nc.dram_tensor("mnr_d", [8, HW], bf16).ap()
    xw_d = nc.dram_tensor("xw_d", [HW], bf16).ap()
    thr_d = nc.dram_tensor("thr_d", [1], f32).ap()

    def dram_bcast(dst_ap, row_ap, scratch_ap, parts):
        """dst[p, :] = row[:] for all p, bounced through DRAM scratch."""
        nc.sync.dma_start(out=scratch_ap, in_=row_ap)
        rep = bass.AP(tensor=scratch_ap.tensor, offset=scratch_ap.offset,
                      ap=[[0, parts]] + [list(d) for d in scratch_ap.ap])
        nc.sync.dma_start(out=dst_ap, in_=rep)

    CH = [(i * 512, min(512, HW - i * 512)) for i in range((HW + 511) // 512)]
    CHC = [(i * 448, 448) for i in range(7)]

    with tile.TileContext(nc) as tc:
        with contextlib.ExitStack() as ctx:
            consts = ctx.enter_context(tc.tile_pool(name="consts", bufs=1))
            xp = ctx.enter_context(tc.tile_pool(name="xp", bufs=2))
            xbp = ctx.enter_context(tc.tile_pool(name="xbp", bufs=2))
            scratch = ctx.enter_context(tc.tile_pool(name="scratch", bufs=2))
            maskp = ctx.enter_context(tc.tile_pool(name="maskp", bufs=1))
            xwbp = ctx.enter_context(tc.tile_pool(name="xwbp", bufs=1))
            bcp = ctx.enter_context(tc.tile_pool(name="bcp", bufs=3))
            mp = ctx.enter_context(tc.tile_pool(name="mp", bufs=1))
            featp = ctx.enter_context(tc.tile_pool(name="featp", bufs=1))
            yp = ctx.enter_context(tc.tile_pool(name="yp", bufs=1))
            smalls = ctx.enter_context(tc.tile_pool(name="smalls", bufs=2))
            statp = ctx.enter_context(tc.tile_pool(name="statp", bufs=1))
            bigps = ctx.enter_context(tc.tile_pool(name="bigps", bufs=1, space="PSUM"))
            convps = ctx.enter_context(tc.tile_pool(name="convps", bufs=1, space="PSUM"))

            # ---- constants ----
            wt_sb = consts.tile([C, 3, OUT], bf16)     # lhsT: [k, ko, m]
            nc.sync.dma_start(out=wt_sb[:], in_=wt_d.rearrange(
                "(ko k) o -> k ko o", k=C))
            cb_sb = consts.tile([C, 3], f32)
            nc.sync.dma_start(out=cb_sb[:], in_=bass.AP(
                tensor=cb_d.tensor, offset=0, ap=[[1, C], [C, 3]]))
            gam_sb = consts.tile([C, 3], f32)
            nc.sync.dma_start(out=gam_sb[:], in_=bass.AP(
                tensor=gam_d.tensor, offset=0, ap=[[1, C], [C, 3]]))
            bet_sb = consts.tile([C, 3], f32)
            nc.sync.dma_start(out=bet_sb[:], in_=bass.AP(
                tensor=bet_d.tensor, offset=0, ap=[[1, C], [C, 3]]))
            # dist-row selector weights: wsel[:, j, :] has ones in column j
            wsel = consts.tile([C, 8, 8], f32)
            nc.vector.memset(wsel[:], 0.0)
            for j in range(8):
                nc.vector.memset(wsel[:, j, j:j + 1], 1.0)
            # mask_sum chunk->row selectors: ms_sel[:, c, :] ones in col c,
            # rows 1..13 only (row 0 of maskall is norm garbage)
            ms_sel = consts.tile([13, 7, 8], bf16)
            nc.vector.memset(ms_sel[:], 0.0)
            for c in range(7):
                nc.vector.memset(ms_sel[:, c, c:c + 1], 1.0)

            for rep in range(n_reps):
                ysum_parts = [statp.tile([C, 2 * len(CH)], f32, tag=f"ysp{m}",
                                         name=f"ysp{m}") for m in range(3)]
                ysq_parts = [statp.tile([C, 2 * len(CH)], f32, tag=f"yqp{m}",
                                        name=f"yqp{m}") for m in range(3)]
                y_sb = [[yp.tile([C, HW], bf16, tag=f"y{b}{m}", name=f"y{b}{m}")
                         for m in range(3)] for b in range(PER_CORE)]

                def stage_load(b, st):
                    st["x_sb"] = xp.tile([C, HW], f32, tag="x", name="x_sb")
                    nc.sync.dma_start(out=st["x_sb"][:], in_=x_d[b])
                    st["x_bf"] = xbp.tile([C, HW], bf16, tag="xbf", name="x_bf")
                    nc.scalar.copy(out=st["x_bf"][:], in_=st["x_sb"][:])

                def stage_dist(b, st):
                    x_sb, x_bf = st["x_sb"], st["x_bf"]
                    # dist psum rows = [norm, s=4..28]; threshold chain is
                    # emitted right after the norm round so it overlaps the
                    # remaining shift rounds
                    for j in range(8):
                        diff = scratch.tile([C, HW], f32, tag="diff", name="diff")
                        if j == 0:
                            x3 = x_sb[:].rearrange("c (h w) -> c h w", w=W)
                            d3 = diff[:].rearrange("c (h w) -> c h w", w=W)
                            for (ho, hl, hi) in ((0, 28, 28), (28, 28, 0)):
                                for (wo, wl, wi) in ((0, 28, 28), (28, 28, 0)):
                                    nc.vector.tensor_sub(
                                        out=d3[:, ho:ho + hl, wo:wo + wl],
                                        in0=x3[:, ho:ho + hl, wo:wo + wl],
                                        in1=x3[:, hi:hi + hl, wi:wi + wl])
                        else:
                            s = UNIQ[j - 1]
                            for (oo, ln, io) in _roll_regions(s * W):
                                nc.vector.tensor_sub(
                                    out=diff[:, oo:oo + ln],
                                    in0=x_sb[:, oo:oo + ln],
                                    in1=x_sb[:, io:io + ln])
                        adiff = scratch.tile([C, HW], f32, tag="adiff",
                                             name="adiff")
                        nc.scalar.activation(out=adiff[:], in_=diff[:],
                                             func=Act.Abs)
                        if j == 0:
                            # norm reduction via the 1-bank conv psum slot:
                            # each chunk is a closed accumulation group that
                            # bn_stats reads AFTER the PE is done with its
                            # bank (the shared-dist-psum variant hangs HW)
                            stats = smalls.tile([1, len(CH), 6], f32,
                                                tag="bnst", name="stats")
                            for ci, (cs, cl) in enumerate(CH):
                                pn = convps.tile([1, 512], f32, tag="py",
                                                 name="pn")
                                nc.tensor.matmul(out=pn[:, 0:cl],
                                                 lhsT=wsel[:, 0, 0:1],
                                                 rhs=adiff[:, cs:cs + cl],
                                                 start=True, stop=True)
                                nc.vector.bn_stats(out=stats[:, ci, :],
                                                   in_=pn[:, 0:cl])
                            # threshold chain overlaps the shift rounds below
                            mv = smalls.tile([1, 2], f32, tag="mv", name="mv")
                            nc.vector.bn_aggr(out=mv[:], in_=stats[:])
                            varu = smalls.tile([1, 1], f32, tag="varu",
                                               name="varu")
                            nc.vector.tensor_scalar_mul(
                                out=varu[:], in0=mv[:, 1:2],
                                scalar1=float(HW) / (HW - 1))
                            r0 = smalls.tile([1, 1], f32, tag="r0", name="r0")
                            nc.scalar.activation(out=r0[:], in_=varu[:],
                                                 func=Act.Sqrt)
                            ri = smalls.tile([1, 1], f32, tag="ri", name="ri")
                            nc.vector.reciprocal(out=ri[:], in_=r0[:])
                            t1 = smalls.tile([1, 1], f32, tag="t1", name="t1")
                            nc.vector.tensor_mul(out=t1[:], in0=varu[:],
                                                 in1=ri[:])
                            nc.vector.tensor_add(out=t1[:], in0=t1[:],
                                                 in1=r0[:])
                            nc.vector.tensor_scalar_mul(out=t1[:], in0=t1[:],
                                                        scalar1=0.5)
                            thresh = smalls.tile([1, 1], f32, tag="thr",
                                                 name="thresh")
                            nc.vector.tensor_sub(out=thresh[:], in0=mv[:, 0:1],
                                                 in1=t1[:])
                            mask0 = smalls.tile([1, 1], f32, tag="m0",
                                                name="mask0")
                            nc.vector.tensor_scalar(out=mask0[:],
                                                    in0=thresh[:],
                                                    scalar1=0.0, scalar2=None,
                                                    op0=Alu.is_gt)
                            thr7 = smalls.tile([7, 1], f32, tag="thr7",
                                               name="thr7")
                            dram_bcast(thr7[:], thresh[0:1, :], thr_d[:], 7)
                            st["mask0"] = mask0
                            pd = bigps.tile([7, 3584], f32, tag="bigpsum",
                                            name="pd")
                        else:
                            for (cs, cl) in CH:
                                nc.tensor.matmul(
                                    out=pd[:, cs:cs + cl],
                                    lhsT=wsel[:, j, 1:8],
                                    rhs=adiff[:, cs:cs + cl],
                                    start=(j == 1), stop=(j == 7))

                    # masks (row j = shift 4(j+1); rows 7..12 = pair rolls)
                    maskall = maskp.tile([13, HW], bf16, tag="maskall",
                                         name="maskall")
                    nc.vector.tensor_scalar(out=maskall[0:7, :],
                                            in0=pd[0:7, 0:HW],
                                            scalar1=thr7[:], scalar2=None,
                                            op0=Alu.is_lt)
                    for i, s in enumerate([4, 8, 12, 16, 20, 24]):
                        msrc = maskall[i:i + 1, :]
                        mdst = maskall[7 + i:8 + i, :]
                        off = s * W
                        nc.sync.dma_start(out=mdst[:, 0:HW - off],
                                          in_=msrc[:, off:HW])
                        nc.sync.dma_start(out=mdst[:, HW - off:HW],
                                          in_=msrc[:, 0:off])
                    mneg = maskp.tile([7, HW], bf16, tag="mneg", name="mneg")
                    nc.vector.tensor_scalar(out=mneg[:], in0=maskall[0:7, :],
                                            scalar1=BIG, scalar2=BIG,
                                            op0=Alu.mult, op1=Alu.subtract)
                    st["mneg"] = mneg

                    # mask_sum -> x_weighted: 448-col chunks land on psum rows
                    # 0..6 (selector lhsT) so stats ops run 7 lanes wide
                    pms = convps.tile([8, 448], f32, tag="py", name="pms")
                    for c in range(7):
                        nc.tensor.matmul(out=pms[:, :],
                                         lhsT=ms_sel[:, c, :],
                                         rhs=maskall[:, c * 448:(c + 1) * 448],
                                         start=(c == 0), stop=(c == 6))
                    mmin7 = smalls.tile([7, 1], f32, tag="mmin7", name="mmin7")
                    mmax7 = smalls.tile([7, 1], f32, tag="mmax7", name="mmax7")
                    nc.vector.tensor_reduce(out=mmin7[:], in_=pms[0:7, :],
                                            axis=AxisX, op=Alu.min)
                    nc.vector.tensor_reduce(out=mmax7[:], in_=pms[0:7, :],
                                            axis=AxisX, op=Alu.max)
                    mm17 = smalls.tile([1, 7, 2], f32, tag="mm17", name="mm17")
                    nc.sync.dma_start(out=mm17[:, :, 0], in_=mmin7[:, 0])
                    nc.sync.dma_start(out=mm17[:, :, 1], in_=mmax7[:, 0])
                    mmin = smalls.tile([1, 1], f32, tag="mmin", name="mmin")
                    mmax = smalls.tile([1, 1], f32, tag="mmax", name="mmax")
                    nc.vector.tensor_reduce(out=mmin[:], in_=mm17[:, :, 0],
                                            axis=AxisX, op=Alu.min)
                    nc.vector.tensor_reduce(out=mmax[:], in_=mm17[:, :, 1],
                                            axis=AxisX, op=Alu.max)
                    # xw threshold: msum < 0.4*(mmax-mmin) - mask0
                    t04 = smalls.tile([1, 1], f32, tag="t04", name="t04")
                    nc.vector.tensor_sub(out=t04[:], in0=mmax[:], in1=mmin[:])
                    nc.vector.tensor_scalar_mul(out=t04[:], in0=t04[:],
                                                scalar1=0.4)
                    nc.vector.tensor_sub(out=t04[:], in0=t04[:],
                                         in1=st["mask0"][:])
                    t047 = smalls.tile([7, 1], f32, tag="t047", name="t047")
                    dram_bcast(t047[:], t04[0:1, :], thr_d[:], 7)
                    xw_rows = maskp.tile([7, 448], bf16, tag="xwrows",
                                         name="xw_rows")
                    nc.vector.tensor_scalar(out=xw_rows[:], in0=pms[0:7, :],
                                            scalar1=t047[:], scalar2=None,
                                            op0=Alu.is_lt)
                    nc.sync.dma_start(
                        out=bass.AP(tensor=xw_d.tensor, offset=0,
                                    ap=[[448, 7], [1, 448]]),
                        in_=xw_rows[:])
                    xw_bc = xwbp.tile([C, HW], bf16, tag="xwbc", name="xw_bc")
                    nc.sync.dma_start(out=xw_bc[:], in_=bass.AP(
                        tensor=xw_d.tensor, offset=0,
                        ap=[[0, C], [1, HW]]))
                    xwx_bf = featp.tile([C, HW], bf16, tag="xwx", name="xwx_bf")
                    nc.vector.tensor_mul(out=xwx_bf[:], in0=st["x_bf"][:],
                                         in1=xw_bc[:])
                    st["xwx_bf"] = xwx_bf

                def stage_m(b, st):
                    x_bf = st["x_bf"]
                    M = mp.tile([C, HW], bf16, tag="M", name="M")
                    tmp = mp.tile([C, HW], bf16, tag="tmp", name="tmp")
                    first = True
                    for j in range(1, 8):
                        mb = bcp.tile([C, HW], bf16, tag="mnbc", name="mb")
                        dram_bcast(mb[:], st["mneg"][j - 1:j, :], mnr_d[j - 1], C)
                        sps = [4 * j] if j == 7 else [4 * j, 56 - 4 * j]
                        for sp in sps:
                            r = sp * W
                            dst = M if first else tmp
                            if sp <= 28:
                                nc.vector.tensor_add(out=dst[:, 0:r],
                                                     in0=x_bf[:, HW - r:HW],
                                                     in1=mb[:, 0:r])
                                nc.vector.tensor_add(out=dst[:, r:HW],
                                                     in0=x_bf[:, 0:HW - r],
                                                     in1=mb[:, r:HW])
                            else:
                                nc.vector.tensor_add(out=dst[:, 0:r],
                                                     in0=x_bf[:, HW - r:HW],
                                                     in1=mb[:, HW - r:HW])
                                nc.vector.tensor_add(out=dst[:, r:HW],
                                                     in0=x_bf[:, 0:HW - r],
                                                     in1=mb[:, 0:HW - r])
                            if not first:
                                nc.vector.tensor_max(out=M[:], in0=M[:],
                                                     in1=tmp[:])
                            first = False
                    xj_bf = featp.tile([C, HW], bf16, tag="xj", name="xj_bf")
                    nc.vector.tensor_sub(out=xj_bf[:], in0=M[:], in1=x_bf[:])
                    nc.vector.tensor_scalar_max(out=xj_bf[:], in0=xj_bf[:],
                                                scalar1=0.0)
                    st["xj_bf"] = xj_bf

                def stage_conv(b, st):
                    feat = [st["x_bf"], st["xj_bf"], st["xwx_bf"]]
                    KO_ORDER = (0, 2, 1)  # xj (ko=1) last: ready latest
                    for ci, (cs, cl) in enumerate(CHC):
                        for m in range(3):
                            py = convps.tile([C, 512], f32, tag="py", name="py")
                            for ki, ko in enumerate(KO_ORDER):
                                nc.tensor.matmul(
                                    out=py[:, 0:cl],
                                    lhsT=wt_sb[:, ko, m * C:(m + 1) * C],
                                    rhs=feat[ko][:, cs:cs + cl],
                                    start=(ki == 0), stop=(ki == 2))
                            idx = b * len(CH) + ci
                            nc.scalar.activation(
                                out=y_sb[b][m][:, cs:cs + cl], in_=py[:, 0:cl],
                                func=Act.Identity, bias=cb_sb[:, m:m + 1],
                                accum_out=ysum_parts[m][:, idx:idx + 1])
                    for m in range(3):
                        sq = scratch.tile([C, HW], bf16, tag="sq", name="sq", bufs=1)
                        nc.scalar.activation(
                            out=sq[:], in_=y_sb[b][m][:], func=Act.Square,
                            accum_out=ysq_parts[m][:, b:b + 1])

                # software-pipelined emission: dist(1) before M(0) keeps the
                # PE fed with matmuls while the DVE runs the max-accumulation
                sts = [dict() for _ in range(PER_CORE)]
                if interleave:
                    stage_load(0, sts[0])
                    stage_dist(0, sts[0])
                    stage_load(1, sts[1])
                    stage_dist(1, sts[1])
                    stage_m(0, sts[0])
                    stage_conv(0, sts[0])
                    stage_m(1, sts[1])
                    stage_conv(1, sts[1])
                else:
                    for b in range(PER_CORE):
                        stage_load(b, sts[b])
                        stage_dist(b, sts[b])
                        stage_m(b, sts[b])
                        stage_conv(b, sts[b])

                # ---- global BN stats (AllReduce over 8 cores) ----
                for m in range(3):
                    ys = smalls.tile([C, 1], f32, tag=f"ys{m}")
                    nc.vector.reduce_sum(out=ys[:], in_=ysum_parts[m][:], axis=AxisX)
                    nc.sync.dma_start(out=cc_in[0, m * C:(m + 1) * C], in_=ys[:, 0])
                    yq = smalls.tile([C, 1], f32, tag=f"yq{m}")
                    nc.vector.reduce_sum(out=yq[:], in_=ysq_parts[m][:], axis=AxisX)
                    nc.sync.dma_start(out=cc_in[1, m * C:(m + 1) * C], in_=yq[:, 0])
                if sim_mode or not use_collective:
                    # TimelineSim has no collective support; timing-equivalent
                    # stand-in (collective latency not modeled either way)
                    nc.sync.dma_start(out=cc_out[:], in_=cc_in[:])
                else:
                    nc.gpsimd.collective_compute(
                        "AllReduce", mybir.AluOpType.add,
                        replica_groups=[list(range(N_CORES))],
                        ins=[cc_in[:]], outs=[cc_out[:]])
                sums = statp.tile([C, 3], f32, tag="sums")
                nc.sync.dma_start(out=sums[:], in_=bass.AP(
                    tensor=cc_out.tensor, offset=0, ap=[[1, C], [C, 3]]))
                sqs = statp.tile([C, 3], f32, tag="sqs")
                nc.sync.dma_start(out=sqs[:], in_=bass.AP(
                    tensor=cc_out.tensor, offset=OUT, ap=[[1, C], [C, 3]]))
                mu = statp.tile([C, 3], f32, tag="mu")
                nc.vector.tensor_scalar_mul(out=mu[:], in0=sums[:], scalar1=1.0 / NTOT)
                ve = statp.tile([C, 3], f32, tag="ve")
                nc.vector.tensor_scalar_mul(out=ve[:], in0=sqs[:], scalar1=1.0 / NTOT)
                t2 = statp.tile([C, 3], f32, tag="t2")
                nc.vector.tensor_mul(out=t2[:], in0=mu[:], in1=mu[:])
                nc.vector.tensor_sub(out=ve[:], in0=ve[:], in1=t2[:])
                nc.vector.tensor_scalar_add(out=ve[:], in0=ve[:], scalar1=BN_EPS)
                rr = statp.tile([C, 3], f32, tag="rr")
                nc.scalar.activation(out=rr[:], in_=ve[:], func=Act.Sqrt)
                rv = statp.tile([C, 3], f32, tag="rv")
                nc.vector.reciprocal(out=rv[:], in_=rr[:])
                nc.vector.tensor_mul(out=rv[:], in0=ve[:], in1=rv[:])
                nc.vector.tensor_add(out=rv[:], in0=rv[:], in1=rr[:])
                nc.vector.tensor_scalar_mul(out=rv[:], in0=rv[:], scalar1=0.5)
                rstd = statp.tile([C, 3], f32, tag="rstd")
                nc.vector.reciprocal(out=rstd[:], in_=rv[:])
                a_sb = statp.tile([C, 3], f32, tag="asb")
                nc.vector.tensor_mul(out=a_sb[:], in0=rstd[:], in1=gam_sb[:])
                b_sb = statp.tile([C, 3], f32, tag="bsb")
                nc.vector.tensor_mul(out=b_sb[:], in0=mu[:], in1=a_sb[:])
                nc.vector.tensor_sub(out=b_sb[:], in0=bet_sb[:], in1=b_sb[:])

                # ---- BN + gelu + store (full rows; staging reuses diff slot) ----
                y3 = y_d.rearrange("b (mo k) hw -> b mo k hw", k=C)
                HH = HW // 2
                for b in range(PER_CORE):
                    for m in range(3):
                        ot = scratch.tile([C, HW], f32, tag="diff", name="ot")
                        for h0 in (0, HH):
                            nc.scalar.activation(out=ot[:, h0:h0 + HH],
                                                 in_=y_sb[b][m][:, h0:h0 + HH],
                                                 func=Act.Gelu,
                                                 bias=b_sb[:, m:m + 1],
                                                 scale=a_sb[:, m:m + 1])
                            nc.sync.dma_start(out=y3[b, m, :, h0:h0 + HH],
                                              in_=ot[:, h0:h0 + HH])

    nc.compile()
    return nc


def _prep_inputs(x, conv_w, conv_b, gamma, beta):
    wt = np.ascontiguousarray(conv_w.T).astype(ml_dtypes.bfloat16)
    maps = []
    for c in range(N_CORES):
        xs = np.ascontiguousarray(
            x[c * PER_CORE:(c + 1) * PER_CORE].reshape(PER_CORE, C, HW))
        maps.append({"x": xs, "wT": wt, "cb": conv_b.astype(np.float32),
                     "gam": gamma.astype(np.float32),
                     "bet": beta.astype(np.float32)})
    return maps


def run_on_cores(nc, in_maps):
    from concourse.bass_utils import run_bass_kernel_spmd
    return run_bass_kernel_spmd(nc, in_maps, list(range(N_CORES)))


def kernel(x, conv_w, conv_b, gamma, beta):
    x = np.asarray(x, dtype=np.float32)
    conv_w = np.asarray(conv_w, dtype=np.float32)
    conv_b = np.asarray(conv_b, dtype=np.float32)
    gamma = np.asarray(gamma, dtype=np.float32)
    beta = np.asarray(beta, dtype=np.float32)
    if "nc" not in _cache:
        _cache["nc"] = build_program(1)
    nc = _cache["nc"]
    in_maps = _prep_inputs(x, conv_w, conv_b, gamma, beta)
    res = run_on_cores(nc, in_maps)
    out = np.concatenate([res.results[i]["y"].reshape(PER_CORE, OUT, H, W)
                          for i in range(N_CORES)], axis=0)
    return out.astype(np.float32)
